# revision 1
# baseline (speedup 1.0000x reference)
"""Trainium2 Bass kernel for nn_DecoderHead (B=2, T=2048, D=1024, H=16, DH=64).

y = x + softmax_causal((x @ Wq.T) reshaped to heads @ k^T / sqrt(D)) @ v

Sharding: 8 cores = 2 (batch) x 4 (head groups of 4 heads). Each core computes
its batch's q-projection for its 256 output features (Wq column-sharded by
head), runs causal attention for its 4 heads, adds the residual slice, and
writes a [T, 256] slice of the output. Host gathers slices (the "all-gather"
over the head-split d dim happens on the host at zero cost).

Per-core kernel layout (all contractions run on the PE partition axis):
  qT[e, t]   = sum_d WqT[d, e] * xT[d, t]          (q projection, transposed)
  sT[tk, tq] = sum_dh kT[dh, tk] * qT_h[dh, tq]    (scores, transposed)
  eT         = exp((sT + causal_mask) / 32)
  oT[dh', tq]= sum_tk vO[tk, dh'] * eT[tk, tq]     (vO = [v | ones] -> row 64
                                                    accumulates the softmax
                                                    denominator)
  out[tq,dh] = transpose(oT) * (1/denom) + x_res   (PE transpose, then one
                                                    fused DVE op per block)
"""

import os
import numpy as np

import concourse.bass as bass
import concourse.mybir as mybir
import concourse.tile as tile
from concourse import bacc
from concourse.alu_op_type import AluOpType
from concourse.bass_utils import run_bass_kernel_spmd

# Problem shape (hardcoded per the harness contract).
B, T, D, H = 2, 2048, 1024, 16
DH = D // H          # 64
N_CORES = 8
HPC = H // (N_CORES // B)   # heads per core = 4
EPC = HPC * DH       # output features per core = 256
P = 128              # SBUF partitions
TQ = 512             # query-tile width (matmul moving-dim)
NTQ = T // TQ        # 4
NTKB = T // P        # 16 key blocks of 128
DT = D // P          # 8 contraction tiles for the q projection
EG = EPC // P        # 2 head-pair groups of 128 e-rows
SCALE = 1.0 / np.sqrt(np.float32(D))   # 1/32, matches reference (sqrt(d))
MASK_NEG = -1.0e9

F32 = mybir.dt.float32

# Matmul operand dtype: "fp32r" (fp32 with 11-bit mantissa, full PE rate),
# "fp32" (exact, 1/4 PE rate), or "bf16".
VARIANT = os.environ.get("DH_VARIANT", "fp32r")


def _mm_dt(variant):
    return {
        "fp32": mybir.dt.float32,
        "fp32r": mybir.dt.float32r,
        "bf16": mybir.dt.bfloat16,
    }[variant]


def _np_round_fp32r(a: np.ndarray) -> np.ndarray:
    """Round fp32 to the fp32r value set: 11-bit mantissa, RNE, low 12 bits 0."""
    u = a.astype(np.float32).view(np.uint32)
    lsb = (u >> np.uint32(12)) & np.uint32(1)
    r = (u + np.uint32(0x7FF) + lsb) & np.uint32(0xFFFFF000)
    return r.view(np.float32)


def _host_cast(a: np.ndarray, variant: str) -> np.ndarray:
    a = np.ascontiguousarray(a, dtype=np.float32)
    if variant == "fp32r":
        return _np_round_fp32r(a)
    if variant == "bf16":
        import ml_dtypes
        return a.astype(ml_dtypes.bfloat16)
    return a


def build_nc(variant: str = VARIANT, repeat: int = 1):
    """Build the per-core SPMD Bass program. `repeat` wraps the whole body in a
    hardware loop (used only for timing measurement)."""
    mdt = _mm_dt(variant)
    nc = bacc.Bacc(
        "TRN2", target_bir_lowering=False, debug=False, num_devices=N_CORES
    )

    # DRAM I/O (per-core shapes; host prepares these layouts)
    xT = nc.dram_tensor("xT", [D, T], mdt, kind="ExternalInput").ap()
    wqT = nc.dram_tensor("wqT", [D, EPC], mdt, kind="ExternalInput").ap()
    kT = nc.dram_tensor("kT", [P, EG, T], mdt, kind="ExternalInput").ap()
    vO = nc.dram_tensor("vO", [P, NTKB, HPC, DH + 1], mdt, kind="ExternalInput").ap()
    xres = nc.dram_tensor("xres", [P, T // P, EPC], F32, kind="ExternalInput").ap()
    maskneg = nc.dram_tensor("maskneg", [P, 896], F32, kind="ExternalInput").ap()
    ident = nc.dram_tensor("ident", [P, P], F32, kind="ExternalInput").ap()
    y = nc.dram_tensor("y", [T, EPC], F32, kind="ExternalOutput").ap()

    with tile.TileContext(nc) as tc:
        with (
            tc.tile_pool(name="const", bufs=1) as cpool,
            tc.tile_pool(name="xq", bufs=1) as xqpool,
            tc.tile_pool(name="work", bufs=3) as wpool,
            tc.tile_pool(name="epi", bufs=2) as epool,
            tc.tile_pool(name="ps_s", bufs=2, space="PSUM") as ps_s,
            tc.tile_pool(name="ps_o", bufs=2, space="PSUM") as ps_o,
            tc.tile_pool(name="ps_t", bufs=2, space="PSUM") as ps_t,
        ):
            def body(_iv=None):
                # ---- loads -------------------------------------------------
                xT_sb = xqpool.tile([P, DT, T], mdt, name="xT_sb", tag="xT_sb")
                for dt_i in range(DT):
                    for tqc in range(NTQ):
                        sl = bass.ts(tqc, TQ)
                        nc.sync.dma_start(
                            xT_sb[:, dt_i, sl], xT[dt_i * P:(dt_i + 1) * P, sl]
                        )
                wq_sb = xqpool.tile([P, DT, EPC], mdt, name="wq_sb", tag="wq_sb")
                for dt_i in range(DT):
                    nc.sync.dma_start(
                        wq_sb[:, dt_i, :], wqT[dt_i * P:(dt_i + 1) * P, :]
                    )
                kT_sb = cpool.tile([P, EG, T], mdt, name="kT_sb", tag="kT_sb")
                for g in range(EG):
                    nc.sync.dma_start(kT_sb[:, g, :], kT[:, g, :])
                vO_sb = cpool.tile([P, NTKB, HPC, DH + 1], mdt, name="vO_sb",
                                   tag="vO_sb")
                for q4 in range(4):
                    nc.sync.dma_start(
                        vO_sb[:, 4 * q4:4 * (q4 + 1)], vO[:, 4 * q4:4 * (q4 + 1)]
                    )
                xr_sb = cpool.tile([P, T // P, EPC], F32, name="xr_sb", tag="xr_sb")
                for q4 in range(4):
                    nc.sync.dma_start(
                        xr_sb[:, 4 * q4:4 * (q4 + 1)], xres[:, 4 * q4:4 * (q4 + 1)]
                    )
                mk_sb = cpool.tile([P, 896], F32, name="mk_sb", tag="mk_sb")
                nc.sync.dma_start(mk_sb[:], maskneg[:])
                id_sb = cpool.tile([P, P], F32, name="id_sb", tag="id_sb")
                nc.sync.dma_start(id_sb[:], ident[:])

                # ---- q projection:  qT[e, t] ------------------------------
                qT_sb = xqpool.tile([P, EG, T], mdt, name="qT_sb", tag="qT_sb")
                for g in range(EG):
                    for tqc in range(NTQ):
                        sl = bass.ts(tqc, TQ)
                        psq = ps_s.tile([P, TQ], F32, name="psq", tag="s")
                        for dt_i in range(DT):
                            nc.tensor.matmul(
                                psq[:],
                                wq_sb[:, dt_i, g * P:(g + 1) * P],
                                xT_sb[:, dt_i, sl],
                                start=(dt_i == 0),
                                stop=(dt_i == DT - 1),
                            )
                        nc.vector.tensor_copy(qT_sb[:, g, sl], psq[:])

                # ---- attention --------------------------------------------
                def epilogue(state):
                    h, tqt, pso_t = state
                    oT = epool.tile([DH + 1, TQ], F32, name="oT", tag="oT")
                    nc.vector.tensor_copy(oT[:], pso_t[:])
                    ysb = epool.tile([P, 4, DH], F32, name="ysb", tag="ysb")
                    for j in range(4):
                        pst = ps_t.tile([P, DH + 1], F32, name="pst", tag="t")
                        nc.tensor.transpose(
                            pst[:],
                            oT[:, j * P:(j + 1) * P],
                            id_sb[0:DH + 1, 0:DH + 1],
                        )
                        rc = epool.tile([P, 1], F32, name="rc", tag="rc", bufs=4)
                        nc.vector.reciprocal(rc[:], pst[:, DH:DH + 1])
                        nc.vector.scalar_tensor_tensor(
                            ysb[:, j, :],
                            pst[:, 0:DH],
                            rc[:],
                            xr_sb[:, 4 * tqt + j, h * DH:(h + 1) * DH],
                            AluOpType.mult,
                            AluOpType.add,
                        )
                    ydst = y[tqt * TQ:(tqt + 1) * TQ, h * DH:(h + 1) * DH]
                    nc.sync.dma_start(
                        ydst.rearrange("(j p) c -> p j c", p=P), ysb[:]
                    )

                pending = None
                for h in range(HPC):
                    g = h // 2
                    bp = DH * (h % 2)
                    for tqt in range(NTQ):
                        ntk = 4 * (tqt + 1)          # valid 128-key blocks
                        npairs = ntk // 2
                        tq_sl = bass.ts(tqt, TQ)
                        pso_t = ps_o.tile([DH + 1, TQ], F32, name="pso", tag="o")
                        prev_et = None
                        for pair in range(npairs):
                            pssc = ps_s.tile([P, 2 * TQ], F32, name="pssc", tag="s")
                            for u in range(2):
                                tkb = 2 * pair + u
                                nc.tensor.matmul(
                                    pssc[:, u * TQ:(u + 1) * TQ],
                                    kT_sb[bp:bp + DH, g, tkb * P:(tkb + 1) * P],
                                    qT_sb[bp:bp + DH, g, tq_sl],
                                    start=True,
                                    stop=True,
                                )
                                off = tkb * P - tqt * TQ
                                if off >= 0:  # diagonal block: additive mask
                                    a0 = 384 - off
                                    nc.vector.tensor_add(
                                        pssc[:, u * TQ:(u + 1) * TQ],
                                        pssc[:, u * TQ:(u + 1) * TQ],
                                        mk_sb[:, a0:a0 + TQ],
                                    )
                            et = wpool.tile([P, 2 * TQ], mdt, name="et", tag="et")
                            nc.scalar.activation(
                                et[:], pssc[:],
                                mybir.ActivationFunctionType.Exp,
                                scale=float(SCALE),
                            )
                            if prev_et is not None:
                                p_et, p_pair = prev_et
                                for u in range(2):
                                    tkb = 2 * p_pair + u
                                    nc.tensor.matmul(
                                        pso_t[:],
                                        vO_sb[:, tkb, h, :],
                                        p_et[:, u * TQ:(u + 1) * TQ],
                                        start=(tkb == 0),
                                        stop=False,
                                    )
                            prev_et = (et, pair)
                            if pair == 0 and pending is not None:
                                epilogue(pending)
                                pending = None
                        p_et, p_pair = prev_et
                        for u in range(2):
                            tkb = 2 * p_pair + u
                            nc.tensor.matmul(
                                pso_t[:],
                                vO_sb[:, tkb, h, :],
                                p_et[:, u * TQ:(u + 1) * TQ],
                                start=(tkb == 0),
                                stop=(u == 1),
                            )
                        pending = (h, tqt, pso_t)
                if pending is not None:
                    epilogue(pending)

            if repeat == 1:
                body()
            else:
                tc.For_i_unrolled(0, repeat, 1, body, max_unroll=1)

    nc.compile()
    return nc


def prep_in_maps(x, k, v, Wq, variant: str = VARIANT):
    """Build the 8 per-core input maps from full inputs (host-side numpy)."""
    x = np.asarray(x, dtype=np.float32)
    k = np.asarray(k, dtype=np.float32)
    v = np.asarray(v, dtype=np.float32)
    Wq = np.asarray(Wq, dtype=np.float32)

    # causal additive mask, shared by all cores: keep iff i <= c - 384
    i_idx = np.arange(P)[:, None]
    c_idx = np.arange(896)[None, :]
    maskneg = np.where(i_idx <= c_idx - 384, 0.0, MASK_NEG).astype(np.float32)
    ident = np.eye(P, dtype=np.float32)

    in_maps = []
    for c in range(N_CORES):
        b = c // (N_CORES // B)
        grp = c % (N_CORES // B)
        heads = slice(HPC * grp, HPC * (grp + 1))
        cols = slice(EPC * grp, EPC * (grp + 1))

        xT_c = x[b].T                                   # [D, T]
        wqT_c = Wq[cols, :].T                           # [D, EPC]
        kT_c = np.zeros((P, EG, T), dtype=np.float32)   # [128, 2, T]
        for lh in range(HPC):
            kT_c[DH * (lh % 2):DH * (lh % 2) + DH, lh // 2, :] = \
                k[b, HPC * grp + lh].T
        vv = v[b, heads]                                # [HPC, T, DH]
        vO_c = np.ones((P, NTKB, HPC, DH + 1), dtype=np.float32)
        vO_c[:, :, :, :DH] = vv.reshape(HPC, NTKB, P, DH).transpose(2, 1, 0, 3)
        xres_c = np.ascontiguousarray(
            x[b][:, cols].reshape(NTKB, P, EPC).transpose(1, 0, 2)
        )
        in_maps.append({
            "xT": _host_cast(xT_c, variant),
            "wqT": _host_cast(wqT_c, variant),
            "kT": _host_cast(kT_c, variant),
            "vO": _host_cast(vO_c, variant),
            "xres": xres_c,
            "maskneg": maskneg,
            "ident": ident,
        })
    return in_maps


def gather_output(results):
    """Assemble full [B, T, D] output from 8 per-core [T, EPC] slices."""
    y = np.empty((B, T, D), dtype=np.float32)
    for c in range(N_CORES):
        b = c // (N_CORES // B)
        grp = c % (N_CORES // B)
        y[b, :, EPC * grp:EPC * (grp + 1)] = results[c]["y"]
    return y


_NC_CACHE = {}


def kernel(x, k, v, Wq):
    key = (VARIANT, 1)
    if key not in _NC_CACHE:
        _NC_CACHE[key] = build_nc(VARIANT, repeat=1)
    nc = _NC_CACHE[key]
    in_maps = prep_in_maps(x, k, v, Wq, VARIANT)
    res = run_bass_kernel_spmd(nc, in_maps, core_ids=list(range(N_CORES)))
    return gather_output(res.results)


# revision 3
# speedup vs baseline: 1.4596x; 1.4596x over previous
"""Trainium2 Bass kernel for nn_DecoderHead (B=2, T=2048, D=1024, H=16, DH=64).

y = x + softmax_causal((x @ Wq.T) split to heads @ k^T / sqrt(D)) @ v

Sharding: 8 cores = 2 (batch) x 4 (head groups of 4 heads). Each core computes
its batch's q-projection for its 256 output features (Wq column-sharded by
head), causal attention for its 4 heads, adds the residual slice, and writes a
[T, 256] slice; the host concatenates slices (the all-gather over the
head-split d dim is a free host-side assembly).

Per-core dataflow (all matmul contractions on the PE partition axis; fp32r
operands give full PE rate with ~11-bit-mantissa rounding):
  qT[e, t]   = sum_d WqT[d, e] * xT[d, t]         (q projection, transposed)
  sT[tk, tq] = sum_dh kT_h[dh, tk] * qT_h[dh, tq] (scores, transposed; two
               heads run concurrently in distinct PE row-groups since DH=64)
  eT         = exp(sT / 32) * causal01            (ACT exp, DVE mask-mul)
  oT[dh', tq]= sum_tk vO[tk, dh'] * eT[tk, tq]    (vO = [v | ones]; row 64
                                                   accumulates the denominator)
  y[tq, dh]  = transpose(oT) / denom + x_res      (PE transpose, DVE fused
                                                   multiply-add epilogue)
"""

import os
from collections import deque

import numpy as np

import concourse.bass as bass
import concourse.mybir as mybir
import concourse.tile as tile
from concourse import bacc
from concourse.alu_op_type import AluOpType
from concourse.bass_utils import run_bass_kernel_spmd

# Problem shape (hardcoded per the harness contract).
B, T, D, H = 2, 2048, 1024, 16
DH = D // H          # 64
N_CORES = 8
HPC = H // (N_CORES // B)   # heads per core = 4
EPC = HPC * DH       # output features per core = 256
P = 128              # SBUF partitions
TQ = 512             # query-tile width (matmul moving-dim)
NTQ = T // TQ        # 4
NTKB = T // P        # 16 key blocks of 128
DT = D // P          # 8 contraction tiles for the q projection
EG = EPC // P        # 2 head-pair groups of 128 e-rows
SCALE = 1.0 / np.sqrt(np.float32(D))   # 1/32 (reference scales by sqrt(d))

F32 = mybir.dt.float32

# Matmul operand dtype: fp32r (fp32 w/ 11-bit mantissa, full PE rate),
# fp32 (exact, 1/4 rate), bf16.
VARIANT = os.environ.get("DH_VARIANT", "fp32r")


def _mm_dt(variant):
    return {
        "fp32": mybir.dt.float32,
        "fp32r": mybir.dt.float32r,
        "bf16": mybir.dt.bfloat16,
    }[variant]


def _np_round_fp32r(a: np.ndarray) -> np.ndarray:
    """Round fp32 to the fp32r value set: 11-bit mantissa, RNE, low 12 bits 0."""
    u = a.astype(np.float32).view(np.uint32)
    lsb = (u >> np.uint32(12)) & np.uint32(1)
    r = (u + np.uint32(0x7FF) + lsb) & np.uint32(0xFFFFF000)
    return r.view(np.float32)


def _host_cast(a: np.ndarray, variant: str) -> np.ndarray:
    a = np.ascontiguousarray(a, dtype=np.float32)
    if variant == "fp32r":
        return _np_round_fp32r(a)
    if variant == "bf16":
        import ml_dtypes
        return a.astype(ml_dtypes.bfloat16)
    return a


def build_nc(variant: str = VARIANT, repeat: int = 1):
    """Build the per-core SPMD Bass program. `repeat` wraps the body in a
    hardware loop (timing only)."""
    mdt = _mm_dt(variant)
    nc = bacc.Bacc(
        "TRN2", target_bir_lowering=False, debug=False, num_devices=N_CORES
    )

    xT = nc.dram_tensor("xT", [D, T], mdt, kind="ExternalInput").ap()
    wqT = nc.dram_tensor("wqT", [D, EPC], mdt, kind="ExternalInput").ap()
    kT = nc.dram_tensor("kT", [P, EG, T], mdt, kind="ExternalInput").ap()
    vO = nc.dram_tensor("vO", [P, NTKB, HPC, DH + 1], mdt, kind="ExternalInput").ap()
    xres = nc.dram_tensor("xres", [P, T // P, EPC], F32, kind="ExternalInput").ap()
    mask3 = nc.dram_tensor("mask3", [P, 4, TQ], F32, kind="ExternalInput").ap()
    ident = nc.dram_tensor("ident", [P, P], F32, kind="ExternalInput").ap()
    y = nc.dram_tensor("y", [T, EPC], F32, kind="ExternalOutput").ap()

    with tile.TileContext(nc) as tc:
        with (
            tc.tile_pool(name="const", bufs=1) as cpool,
            tc.tile_pool(name="xq", bufs=1) as xqpool,
            tc.tile_pool(name="work", bufs=4) as wpool,
            tc.tile_pool(name="epi", bufs=2) as epool,
            tc.tile_pool(name="ps_s", bufs=2, space="PSUM") as ps_s,
            tc.tile_pool(name="ps_o", bufs=2, space="PSUM") as ps_o,
            tc.tile_pool(name="ps_t", bufs=2, space="PSUM") as ps_t,
        ):
            def body(_iv=None):
                # ---- loads (wq + xT first: qproj gates everything) --------
                wq_sb = xqpool.tile([P, DT, EPC], mdt, name="wq_sb", tag="wq_sb")
                for dt_i in range(DT):
                    nc.sync.dma_start(
                        wq_sb[:, dt_i, :], wqT[dt_i * P:(dt_i + 1) * P, :]
                    )
                xT_sb = xqpool.tile([P, DT, T], mdt, name="xT_sb", tag="xT_sb")
                for tqc in range(NTQ):
                    sl = bass.ts(tqc, TQ)
                    for dt_i in range(DT):
                        nc.sync.dma_start(
                            xT_sb[:, dt_i, sl], xT[dt_i * P:(dt_i + 1) * P, sl]
                        )
                kT_sb = cpool.tile([P, EG, T], mdt, name="kT_sb", tag="kT_sb")
                for g in range(EG):
                    nc.sync.dma_start(kT_sb[:, g, :], kT[:, g, :])
                vO_sb = cpool.tile([P, NTKB, HPC, DH + 1], mdt, name="vO_sb",
                                   tag="vO_sb")
                for q4 in range(4):
                    nc.sync.dma_start(
                        vO_sb[:, 4 * q4:4 * (q4 + 1)], vO[:, 4 * q4:4 * (q4 + 1)]
                    )
                mk_sb = cpool.tile([P, 4, TQ], F32, name="mk_sb", tag="mk_sb")
                nc.sync.dma_start(mk_sb[:], mask3[:])
                id_sb = cpool.tile([P, P], F32, name="id_sb", tag="id_sb")
                nc.sync.dma_start(id_sb[:], ident[:])
                xr_sb = cpool.tile([P, T // P, EPC], F32, name="xr_sb", tag="xr_sb")
                for q4 in range(4):
                    nc.sync.dma_start(
                        xr_sb[:, 4 * q4:4 * (q4 + 1)], xres[:, 4 * q4:4 * (q4 + 1)]
                    )

                # ---- q projection (tq-major so attention starts early) ----
                qT_sb = xqpool.tile([P, EG, T], mdt, name="qT_sb", tag="qT_sb")
                for tqc in range(NTQ):
                    sl = bass.ts(tqc, TQ)
                    for g in range(EG):
                        psq = ps_s.tile([P, TQ], F32, name="psq", tag="s")
                        for dt_i in range(DT):
                            nc.tensor.matmul(
                                psq[:],
                                wq_sb[:, dt_i, g * P:(g + 1) * P],
                                xT_sb[:, dt_i, sl],
                                start=(dt_i == 0),
                                stop=(dt_i == DT - 1),
                            )
                        nc.scalar.copy(qT_sb[:, g, sl], psq[:])

                # ---- attention: head pairs share PE via row-groups --------
                pending = deque()

                def epilogue(state):
                    h, tqt, pso_t = state
                    oT = epool.tile([DH + 1, TQ], F32, name="oT", tag="oT")
                    nc.vector.tensor_copy(oT[:], pso_t[:])
                    ysb = epool.tile([P, 4, DH], F32, name="ysb", tag="ysb")
                    for j in range(4):
                        pst = ps_t.tile([P, DH + 1], F32, name="pst", tag="t")
                        nc.tensor.transpose(
                            pst[:],
                            oT[:, j * P:(j + 1) * P],
                            id_sb[0:DH + 1, 0:DH + 1],
                        )
                        rc = epool.tile([P, 1], F32, name="rc", tag="rc", bufs=4)
                        nc.vector.reciprocal(rc[:], pst[:, DH:DH + 1])
                        nc.vector.scalar_tensor_tensor(
                            ysb[:, j, :],
                            pst[:, 0:DH],
                            rc[:],
                            xr_sb[:, 4 * tqt + j, h * DH:(h + 1) * DH],
                            AluOpType.mult,
                            AluOpType.add,
                        )
                    ydst = y[tqt * TQ:(tqt + 1) * TQ, h * DH:(h + 1) * DH]
                    nc.sync.dma_start(
                        ydst.rearrange("(j p) c -> p j c", p=P), ysb[:]
                    )

                for hp in range(HPC // 2):
                    g = hp
                    for tqt in range(NTQ):
                        ntk = 4 * (tqt + 1)
                        npairs = ntk // 2
                        tq_sl = bass.ts(tqt, TQ)
                        pso2 = [
                            ps_o.tile([DH + 1, TQ], F32, name=f"pso{i}", tag="o")
                            for i in range(2)
                        ]
                        prev = None
                        for pair in range(npairs):
                            et2 = []
                            for i in range(2):   # i: head within pair
                                bp = DH * i
                                pssc = ps_s.tile([P, 2, TQ], F32,
                                                 name=f"pssc{i}", tag="s")
                                for u in range(2):
                                    tkb = 2 * pair + u
                                    nc.tensor.matmul(
                                        pssc[:, u, :],
                                        kT_sb[bp:bp + DH, g,
                                              tkb * P:(tkb + 1) * P],
                                        qT_sb[bp:bp + DH, g, tq_sl],
                                        start=True,
                                        stop=True,
                                    )
                                et = wpool.tile([P, 2, TQ], mdt,
                                                name=f"et{i}", tag="et")
                                nc.scalar.activation(
                                    et[:], pssc[:],
                                    mybir.ActivationFunctionType.Exp,
                                    scale=float(SCALE),
                                )
                                if 2 * pair >= 4 * tqt:   # diagonal pair
                                    m0 = 2 * pair - 4 * tqt
                                    nc.vector.tensor_mul(
                                        et[:], et[:], mk_sb[:, m0:m0 + 2, :]
                                    )
                                et2.append(et)
                            if prev is not None:
                                p_et2, p_pair = prev
                                for i in range(2):
                                    for u in range(2):
                                        tkb = 2 * p_pair + u
                                        nc.tensor.matmul(
                                            pso2[i][:],
                                            vO_sb[:, tkb, 2 * hp + i, :],
                                            p_et2[i][:, u, :],
                                            start=(tkb == 0),
                                            stop=False,
                                        )
                            prev = (et2, pair)
                            if pair == 0:
                                while pending:
                                    epilogue(pending.popleft())
                        p_et2, p_pair = prev
                        for i in range(2):
                            for u in range(2):
                                tkb = 2 * p_pair + u
                                nc.tensor.matmul(
                                    pso2[i][:],
                                    vO_sb[:, tkb, 2 * hp + i, :],
                                    p_et2[i][:, u, :],
                                    start=(tkb == 0),
                                    stop=(u == 1),
                                )
                        for i in range(2):
                            pending.append((2 * hp + i, tqt, pso2[i]))
                while pending:
                    epilogue(pending.popleft())

            if repeat == 1:
                body()
            else:
                tc.For_i_unrolled(0, repeat, 1, body, max_unroll=1)

    nc.compile()
    return nc


def prep_in_maps(x, k, v, Wq, variant: str = VARIANT):
    """Build the 8 per-core input maps from full inputs (host-side numpy)."""
    x = np.asarray(x, dtype=np.float32)
    k = np.asarray(k, dtype=np.float32)
    v = np.asarray(v, dtype=np.float32)
    Wq = np.asarray(Wq, dtype=np.float32)

    # mask3[i, m, j] = 1 where key i (+128m block offset) is visible to query j
    i_idx = np.arange(P)[:, None, None]
    m_idx = np.arange(4)[None, :, None]
    j_idx = np.arange(TQ)[None, None, :]
    mask3 = (i_idx + P * m_idx <= j_idx).astype(np.float32)
    ident = np.eye(P, dtype=np.float32)

    in_maps = []
    for c in range(N_CORES):
        b = c // (N_CORES // B)
        grp = c % (N_CORES // B)
        heads = slice(HPC * grp, HPC * (grp + 1))
        cols = slice(EPC * grp, EPC * (grp + 1))

        xT_c = x[b].T                                   # [D, T]
        wqT_c = Wq[cols, :].T                           # [D, EPC]
        kT_c = np.zeros((P, EG, T), dtype=np.float32)
        for lh in range(HPC):
            kT_c[DH * (lh % 2):DH * (lh % 2) + DH, lh // 2, :] = \
                k[b, HPC * grp + lh].T
        vv = v[b, heads]                                # [HPC, T, DH]
        vO_c = np.ones((P, NTKB, HPC, DH + 1), dtype=np.float32)
        vO_c[:, :, :, :DH] = vv.reshape(HPC, NTKB, P, DH).transpose(2, 1, 0, 3)
        xres_c = np.ascontiguousarray(
            x[b][:, cols].reshape(NTKB, P, EPC).transpose(1, 0, 2)
        )
        in_maps.append({
            "xT": _host_cast(xT_c, variant),
            "wqT": _host_cast(wqT_c, variant),
            "kT": _host_cast(kT_c, variant),
            "vO": _host_cast(vO_c, variant),
            "xres": xres_c,
            "mask3": mask3,
            "ident": ident,
        })
    return in_maps


def gather_output(results):
    """Assemble full [B, T, D] output from 8 per-core [T, EPC] slices."""
    y = np.empty((B, T, D), dtype=np.float32)
    for c in range(N_CORES):
        b = c // (N_CORES // B)
        grp = c % (N_CORES // B)
        y[b, :, EPC * grp:EPC * (grp + 1)] = results[c]["y"]
    return y


_NC_CACHE = {}


def kernel(x, k, v, Wq):
    key = (VARIANT, 1)
    if key not in _NC_CACHE:
        _NC_CACHE[key] = build_nc(VARIANT, repeat=1)
    nc = _NC_CACHE[key]
    in_maps = prep_in_maps(x, k, v, Wq, VARIANT)
    res = run_bass_kernel_spmd(nc, in_maps, core_ids=list(range(N_CORES)))
    return gather_output(res.results)


# revision 5
# speedup vs baseline: 1.6921x; 1.1593x over previous
"""Trainium2 Bass kernel for nn_DecoderHead (B=2, T=2048, D=1024, H=16, DH=64).

y = x + softmax_causal((x @ Wq.T) split to heads @ k^T / sqrt(D)) @ v

Sharding: 8 cores = 2 (batch) x 4 (head groups of 4 heads). Each core computes
its batch's q-projection for its 256 output features (Wq column-sharded by
head), causal attention for its 4 heads, adds the residual slice, and writes a
[T, 256] slice; the host concatenates slices (the all-gather over the
head-split d dim is a free host-side assembly).

Per-core dataflow (all matmul contractions on the PE partition axis; fp32r
operands give full PE rate with ~11-bit-mantissa input rounding):
  qT[e, t]   = sum_d WqT[d, e] * xT[d, t]         (q projection, transposed)
  sT[tk, tq] = sum_dh kT_h[dh, tk] * qT_h[dh, tq] (scores, transposed; two
               heads run concurrently in distinct PE row-groups since DH=64)
  eT         = exp(sT / 32) * causal01            (ACT exp, DVE mask-mul on
                                                   diagonal blocks only)
  oT[dh', tq]= sum_tk vO[tk, dh'] * eT[tk, tq]    (vO = [v | ones]; row 64
                                                   accumulates the denominator)
  y[tq, dh]  = transpose(oT) / denom + x_res      (PE transpose into one PSUM
                                                   bank, fused DVE epilogue)

The whole schedule is tq-tile-major: load stage c's inputs, project q for tile
c, then run both head-pairs' attention for tile c — so compute starts after
~4 MB of DMA instead of the full 15.5 MB.
"""

import os
from collections import deque

import numpy as np

import concourse.bass as bass
import concourse.mybir as mybir
import concourse.tile as tile
from concourse import bacc
from concourse.alu_op_type import AluOpType
from concourse.bass_utils import run_bass_kernel_spmd

# Problem shape (hardcoded per the harness contract).
B, T, D, H = 2, 2048, 1024, 16
DH = D // H          # 64
N_CORES = 8
HPC = H // (N_CORES // B)   # heads per core = 4
EPC = HPC * DH       # output features per core = 256
P = 128              # SBUF partitions
TQ = 512             # query-tile width (matmul moving-dim)
NTQ = T // TQ        # 4
NTKB = T // P        # 16 key blocks of 128
DT = D // P          # 8 contraction tiles for the q projection
EG = EPC // P        # 2 head-pair groups of 128 e-rows
SCALE = 1.0 / np.sqrt(np.float32(D))   # 1/32 (reference scales by sqrt(d))

F32 = mybir.dt.float32

# Matmul operand dtype: fp32r (fp32 w/ 11-bit mantissa, full PE rate),
# fp32 (exact, 1/4 rate), bf16.
VARIANT = os.environ.get("DH_VARIANT", "fp32r")


def _mm_dt(variant):
    return {
        "fp32": mybir.dt.float32,
        "fp32r": mybir.dt.float32r,
        "bf16": mybir.dt.bfloat16,
    }[variant]


def _np_round_fp32r(a: np.ndarray) -> np.ndarray:
    """Round fp32 to the fp32r value set: 11-bit mantissa, RNE, low 12 bits 0."""
    u = a.astype(np.float32).view(np.uint32)
    lsb = (u >> np.uint32(12)) & np.uint32(1)
    r = (u + np.uint32(0x7FF) + lsb) & np.uint32(0xFFFFF000)
    return r.view(np.float32)


def _host_cast(a: np.ndarray, variant: str) -> np.ndarray:
    a = np.ascontiguousarray(a, dtype=np.float32)
    if variant == "fp32r":
        return _np_round_fp32r(a)
    if variant == "bf16":
        import ml_dtypes
        return a.astype(ml_dtypes.bfloat16)
    return a


def build_nc(variant: str = VARIANT, repeat: int = 1):
    """Build the per-core SPMD Bass program. `repeat` wraps the body in a
    hardware loop (timing only)."""
    mdt = _mm_dt(variant)
    nc = bacc.Bacc(
        "TRN2", target_bir_lowering=False, debug=False, num_devices=N_CORES
    )

    xT = nc.dram_tensor("xT", [D, T], mdt, kind="ExternalInput").ap()
    wqT = nc.dram_tensor("wqT", [D, EPC], mdt, kind="ExternalInput").ap()
    kT = nc.dram_tensor("kT", [P, EG, T], mdt, kind="ExternalInput").ap()
    vO = nc.dram_tensor("vO", [P, NTKB, HPC, DH + 1], mdt, kind="ExternalInput").ap()
    xres = nc.dram_tensor("xres", [P, T // P, EPC], F32, kind="ExternalInput").ap()
    mask3 = nc.dram_tensor("mask3", [P, 4, TQ], F32, kind="ExternalInput").ap()
    ident = nc.dram_tensor("ident", [P, P], F32, kind="ExternalInput").ap()
    y = nc.dram_tensor("y", [T, EPC], F32, kind="ExternalOutput").ap()

    with tile.TileContext(nc) as tc:
        with (
            tc.tile_pool(name="const", bufs=1) as cpool,
            tc.tile_pool(name="xq", bufs=1) as xqpool,
            tc.tile_pool(name="work", bufs=4) as wpool,
            tc.tile_pool(name="epi", bufs=2) as epool,
            tc.tile_pool(name="ps_s", bufs=2, space="PSUM") as ps_s,
            tc.tile_pool(name="ps_o", bufs=2, space="PSUM") as ps_o,
            tc.tile_pool(name="ps_t", bufs=2, space="PSUM") as ps_t,
        ):
            def body(_iv=None):
                # ---- tiles -------------------------------------------------
                id_sb = cpool.tile([P, P], F32, name="id_sb", tag="id_sb")
                mk_sb = cpool.tile([P, 4, TQ], F32, name="mk_sb", tag="mk_sb")
                wq_sb = xqpool.tile([P, DT, EPC], mdt, name="wq_sb", tag="wq_sb")
                xT_sb = xqpool.tile([P, DT, T], mdt, name="xT_sb", tag="xT_sb")
                kT_sb = cpool.tile([P, EG, T], mdt, name="kT_sb", tag="kT_sb")
                vO_sb = cpool.tile([P, NTKB, HPC, DH + 1], mdt, name="vO_sb",
                                   tag="vO_sb")
                xr_sb = cpool.tile([P, T // P, EPC], F32, name="xr_sb",
                                   tag="xr_sb")
                qT_sb = xqpool.tile([P, EG, T], mdt, name="qT_sb", tag="qT_sb")

                # ---- stage-0 loads ----------------------------------------
                nc.sync.dma_start(id_sb[:], ident[:])
                for dt_i in range(DT):
                    nc.sync.dma_start(
                        wq_sb[:, dt_i, :], wqT[dt_i * P:(dt_i + 1) * P, :]
                    )

                def load_stage(c):
                    """Inputs first needed by tq-tile c."""
                    sl = bass.ts(c, TQ)
                    for dt_i in range(DT):
                        nc.sync.dma_start(
                            xT_sb[:, dt_i, sl], xT[dt_i * P:(dt_i + 1) * P, sl]
                        )
                    for g in range(EG):
                        nc.sync.dma_start(kT_sb[:, g, sl], kT[:, g, sl])
                    nc.sync.dma_start(
                        vO_sb[:, 4 * c:4 * (c + 1)], vO[:, 4 * c:4 * (c + 1)]
                    )
                    nc.sync.dma_start(
                        xr_sb[:, 4 * c:4 * (c + 1)], xres[:, 4 * c:4 * (c + 1)]
                    )

                load_stage(0)
                nc.sync.dma_start(mk_sb[:], mask3[:])

                # Warm-up while stage-0 DMA streams: prime the ACT exp table
                # and keep PE busy so the HAM clock-gate opens (dummy work on
                # the identity tile; results unused).
                warm_et = wpool.tile([P, P], F32, name="warm_et", tag="warm")
                psw = ps_o.tile([P, P], F32, name="psw", tag="o")
                for w in range(12):
                    nc.tensor.matmul(
                        psw[:], id_sb[:], id_sb[:], start=True, stop=True,
                    )
                nc.scalar.activation(
                    warm_et[:], psw[:],
                    mybir.ActivationFunctionType.Exp, scale=0.01,
                )

                pending = deque()

                def epilogue(state):
                    h, tqt, pso_t = state
                    oT = epool.tile([DH + 1, TQ], F32, name="oT", tag="oT")
                    nc.vector.tensor_copy(oT[:], pso_t[:])
                    ysb = epool.tile([P, 4, DH], F32, name="ysb", tag="ysb")
                    pst = ps_t.tile([P, 4, DH + 1], F32, name="pst", tag="t")
                    for j in range(4):
                        nc.tensor.transpose(
                            pst[:, j, :],
                            oT[:, j * P:(j + 1) * P],
                            id_sb[0:DH + 1, 0:DH + 1],
                        )
                    rc = epool.tile([P, 4], F32, name="rc", tag="rc", bufs=4)
                    nc.vector.reciprocal(rc[:], pst[:, :, DH])
                    for j in range(4):
                        nc.vector.scalar_tensor_tensor(
                            ysb[:, j, :],
                            pst[:, j, 0:DH],
                            rc[:, j:j + 1],
                            xr_sb[:, 4 * tqt + j, h * DH:(h + 1) * DH],
                            AluOpType.mult,
                            AluOpType.add,
                        )
                    ydst = y[tqt * TQ:(tqt + 1) * TQ, h * DH:(h + 1) * DH]
                    nc.sync.dma_start(
                        ydst.rearrange("(j p) c -> p j c", p=P), ysb[:]
                    )

                def attention(hp, tqt):
                    g = hp
                    ntk = 4 * (tqt + 1)
                    npairs = ntk // 2
                    tq_sl = bass.ts(tqt, TQ)
                    pso2 = [
                        ps_o.tile([DH + 1, TQ], F32, name=f"pso{i}", tag="o")
                        for i in range(2)
                    ]
                    prev = None
                    for pair in range(npairs):
                        et2 = []
                        for i in range(2):   # head within the pair
                            bp = DH * i
                            pssc = ps_s.tile([P, 2, TQ], F32,
                                             name=f"pssc{i}", tag="s")
                            for u in range(2):
                                tkb = 2 * pair + u
                                nc.tensor.matmul(
                                    pssc[:, u, :],
                                    kT_sb[bp:bp + DH, g,
                                          tkb * P:(tkb + 1) * P],
                                    qT_sb[bp:bp + DH, g, tq_sl],
                                    start=True,
                                    stop=True,
                                )
                            et = wpool.tile([P, 2, TQ], mdt,
                                            name=f"et{i}", tag="et")
                            nc.scalar.activation(
                                et[:], pssc[:],
                                mybir.ActivationFunctionType.Exp,
                                scale=float(SCALE),
                            )
                            if 2 * pair >= 4 * tqt:   # diagonal pair
                                m0 = 2 * pair - 4 * tqt
                                nc.vector.tensor_mul(
                                    et[:], et[:], mk_sb[:, m0:m0 + 2, :]
                                )
                            et2.append(et)
                        if prev is not None:
                            p_et2, p_pair = prev
                            for i in range(2):
                                for u in range(2):
                                    tkb = 2 * p_pair + u
                                    nc.tensor.matmul(
                                        pso2[i][:],
                                        vO_sb[:, tkb, 2 * hp + i, :],
                                        p_et2[i][:, u, :],
                                        start=(tkb == 0),
                                        stop=False,
                                    )
                        prev = (et2, pair)
                        if pair == 0:
                            while pending:
                                epilogue(pending.popleft())
                    p_et2, p_pair = prev
                    for i in range(2):
                        for u in range(2):
                            tkb = 2 * p_pair + u
                            nc.tensor.matmul(
                                pso2[i][:],
                                vO_sb[:, tkb, 2 * hp + i, :],
                                p_et2[i][:, u, :],
                                start=(tkb == 0),
                                stop=(u == 1),
                            )
                    for i in range(2):
                        pending.append((2 * hp + i, tqt, pso2[i]))

                # ---- main schedule: tq-tile-major -------------------------
                for tqc in range(NTQ):
                    if tqc + 1 < NTQ:
                        load_stage(tqc + 1)
                    sl = bass.ts(tqc, TQ)
                    for g in range(EG):
                        psq = ps_s.tile([P, TQ], F32, name="psq", tag="s")
                        for dt_i in range(DT):
                            nc.tensor.matmul(
                                psq[:],
                                wq_sb[:, dt_i, g * P:(g + 1) * P],
                                xT_sb[:, dt_i, sl],
                                start=(dt_i == 0),
                                stop=(dt_i == DT - 1),
                            )
                        nc.vector.tensor_copy(qT_sb[:, g, sl], psq[:])
                    for hp in range(HPC // 2):
                        attention(hp, tqc)
                while pending:
                    epilogue(pending.popleft())

            if repeat == 1:
                body()
            else:
                tc.For_i_unrolled(0, repeat, 1, body, max_unroll=1)

    nc.compile()
    return nc


def prep_in_maps(x, k, v, Wq, variant: str = VARIANT):
    """Build the 8 per-core input maps from full inputs (host-side numpy)."""
    x = np.asarray(x, dtype=np.float32)
    k = np.asarray(k, dtype=np.float32)
    v = np.asarray(v, dtype=np.float32)
    Wq = np.asarray(Wq, dtype=np.float32)

    # mask3[i, m, j] = 1 where key i (+128m block offset) is visible to query j
    i_idx = np.arange(P)[:, None, None]
    m_idx = np.arange(4)[None, :, None]
    j_idx = np.arange(TQ)[None, None, :]
    mask3 = (i_idx + P * m_idx <= j_idx).astype(np.float32)
    ident = np.eye(P, dtype=np.float32)

    in_maps = []
    for c in range(N_CORES):
        b = c // (N_CORES // B)
        grp = c % (N_CORES // B)
        heads = slice(HPC * grp, HPC * (grp + 1))
        cols = slice(EPC * grp, EPC * (grp + 1))

        xT_c = x[b].T                                   # [D, T]
        wqT_c = Wq[cols, :].T                           # [D, EPC]
        kT_c = np.zeros((P, EG, T), dtype=np.float32)
        for lh in range(HPC):
            kT_c[DH * (lh % 2):DH * (lh % 2) + DH, lh // 2, :] = \
                k[b, HPC * grp + lh].T
        vv = v[b, heads]                                # [HPC, T, DH]
        vO_c = np.ones((P, NTKB, HPC, DH + 1), dtype=np.float32)
        vO_c[:, :, :, :DH] = vv.reshape(HPC, NTKB, P, DH).transpose(2, 1, 0, 3)
        xres_c = np.ascontiguousarray(
            x[b][:, cols].reshape(NTKB, P, EPC).transpose(1, 0, 2)
        )
        in_maps.append({
            "xT": _host_cast(xT_c, variant),
            "wqT": _host_cast(wqT_c, variant),
            "kT": _host_cast(kT_c, variant),
            "vO": _host_cast(vO_c, variant),
            "xres": xres_c,
            "mask3": mask3,
            "ident": ident,
        })
    return in_maps


def gather_output(results):
    """Assemble full [B, T, D] output from 8 per-core [T, EPC] slices."""
    y = np.empty((B, T, D), dtype=np.float32)
    for c in range(N_CORES):
        b = c // (N_CORES // B)
        grp = c % (N_CORES // B)
        y[b, :, EPC * grp:EPC * (grp + 1)] = results[c]["y"]
    return y


_NC_CACHE = {}


def kernel(x, k, v, Wq):
    key = (VARIANT, 1)
    if key not in _NC_CACHE:
        _NC_CACHE[key] = build_nc(VARIANT, repeat=1)
    nc = _NC_CACHE[key]
    in_maps = prep_in_maps(x, k, v, Wq, VARIANT)
    res = run_bass_kernel_spmd(nc, in_maps, core_ids=list(range(N_CORES)))
    return gather_output(res.results)
